# revision 6
# baseline (speedup 1.0000x reference)
"""GQA attention (B=2,S=2048,D=2048,H=16,KV=4,HD=128) + RoPE on 8 TRN2 NeuronCores.

Sharding: core c -> (batch b=c//4, kv-group g=c%4). Each core projects
Q (4 heads), K/V (1 kv head) for its batch from a replicated x^T, applies
RoPE, runs causal flash attention (scores^T layout, no-max softmax --
|scores|<9 so fp32 exp is safe), AllGathers the per-head attention outputs
across the 4-core batch group, and computes a column slice of the output
projection (column-parallel wo).

The PE is the bottleneck (GPIO power throttle caps it at 13/16 clock):
- softmax denominators are folded off the PE: exp'd score tiles are summed
  on the VectorE (bf16 adds) + one ones-matmul per (head, chunk).
- DMA priority: per-DGE-queue throughput is low (~40-90GB/s) and the 16
  SDMA engines share ~260GB/s, so each queue leads with its first-needed
  bytes; only the gpsimd-queue tail (mask/late rope tables) is dep-gated.
- AllGather runs per head (not per head-pair) and the output projection
  reads quarter rhs tiles, so the final chunk's projection overlaps the
  final attention chunk instead of serializing after the last gather.

Host-side prep (inside kernel()): transpose/cast inputs to bf16 (x in
chunk-major layout for 4KB DMA descriptors), expand RoPE tables, build
permutation/identity/mask constants. Host-side post: transpose +
concatenate the 8 output column-slices.
"""
import numpy as np
import ml_dtypes

import concourse.bass as bass
import concourse.mybir as mybir
import concourse.tile as tile
from concourse import bacc
from concourse.bass import ts
from concourse.bass_utils import run_bass_kernel_spmd
from concourse.tile_rust import add_dep_helper

BF = mybir.dt.bfloat16
F32 = mybir.dt.float32
bf16 = ml_dtypes.bfloat16

B, S, D = 2, 2048, 2048
H, KV, HD = 16, 4, 128
NT = 4          # 512-token chunks
ND = 16         # 128-wide D chunks
NH = 4          # heads per core
SCALE = 1.0 / np.sqrt(HD)
RG = [[0, 1, 2, 3], [4, 5, 6, 7]]
# projection chain order: m=0 K, m=1 V, m=2..5 Q heads 0..3
M_K, M_V, M_Q0 = 0, 1, 2


def build_nc():
    nc = bacc.Bacc("TRN2", target_bir_lowering=False, debug=False, num_devices=8)
    xt_d = nc.dram_tensor("xt", [NT, 128, ND, 512], BF, kind="ExternalInput").ap()
    wqkv_d = nc.dram_tensor("wqkvT", [6, 128, ND, 128], BF, kind="ExternalInput").ap()
    woT_d = nc.dram_tensor("woT", [4, 128, ND, 128], BF, kind="ExternalInput").ap()
    cos_d = nc.dram_tensor("cose", [128, S], BF, kind="ExternalInput").ap()
    sin_d = nc.dram_tensor("sins", [128, S], BF, kind="ExternalInput").ap()
    mask_d = nc.dram_tensor("mask01", [128, 512], BF, kind="ExternalInput").ap()
    pswap_d = nc.dram_tensor("pswap", [128, 128], BF, kind="ExternalInput").ap()
    ident_d = nc.dram_tensor("ident", [128, 128], BF, kind="ExternalInput").ap()
    onesc_d = nc.dram_tensor("onesc", [128, 128], BF, kind="ExternalInput").ap()
    out_d = nc.dram_tensor("out", [512, S], BF, kind="ExternalOutput").ap()

    with tile.TileContext(nc) as tc:
        with (
            tc.tile_pool(name="consts", bufs=1) as consts,
            tc.tile_pool(name="io", bufs=2) as io,
            tc.tile_pool(name="work", bufs=3) as work,
            tc.tile_pool(name="psS", bufs=3, space="PSUM") as psS,
            tc.tile_pool(name="psA", bufs=3, space="PSUM") as psA,
            tc.tile_pool(name="psB", bufs=2, space="PSUM") as psB,
            tc.tile_pool(name="dram", bufs=1, space="DRAM") as dram,
        ):
            # ---- persistent SBUF; DMA emit order = availability order on the
            # gpsimd DGE queue. Critical-first: K weights (split), chunk-0
            # rope tables, pswap, V weights + ident, Q weights. Everything
            # else is emitted now but gated behind early matmuls (deps wired
            # after proj_chunk(0) builds) so it doesn't steal early HBM bw.
            # per-queue throughput is only ~40-90GB/s, so each of the three
            # DGE queues (gpsimd/sync/scalar) leads with the bytes its first
            # consumers need; x chunks are posted upfront (io bufs=4 -> no
            # slot waits) at queue tails so nothing time-sensitive sits
            # behind them.
            w_sb = consts.tile([128, 6, ND, 128], BF, name="w_sb")
            cos_sb = consts.tile([128, S], BF, name="cos_sb")
            sin_sb = consts.tile([128, S], BF, name="sin_sb")
            xt_tiles = [io.tile([128, ND, 512], BF, tag="io512",
                                name=f"xt_t{i}", bufs=4) for i in range(NT)]
            # gpsimd: K weights (split), pswap, Q0/Q1 weights
            nc.gpsimd.dma_start(w_sb[:, M_K, :8], wqkv_d[M_K, :, :8])
            nc.gpsimd.dma_start(w_sb[:, M_K, 8:], wqkv_d[M_K, :, 8:])
            pswap_sb = consts.tile([128, 128], BF, name="pswap_sb")
            nc.gpsimd.dma_start(pswap_sb, pswap_d)
            for m in (M_Q0, M_Q0 + 1):
                nc.gpsimd.dma_start(w_sb[:, m], wqkv_d[m])
            # scalar: chunk-0 rope tables, x0 middle quarters, Q2/Q3, x2, x3
            nc.scalar.dma_start(cos_sb[:, ts(0, 512)], cos_d[:, ts(0, 512)])
            nc.scalar.dma_start(sin_sb[:, ts(0, 512)], sin_d[:, ts(0, 512)])
            nc.scalar.dma_start(xt_tiles[0][:, 4:8, :], xt_d[0, :, 4:8, :])
            nc.scalar.dma_start(xt_tiles[0][:, 8:12, :], xt_d[0, :, 8:12, :])
            for m in (M_Q0 + 2, M_Q0 + 3):
                nc.scalar.dma_start(w_sb[:, m], wqkv_d[m])
            nc.scalar.dma_start(xt_tiles[2], xt_d[2])
            nc.scalar.dma_start(xt_tiles[3], xt_d[3])
            # sync: x0 outer quarters, V weights, ident, x1, wo
            nc.sync.dma_start(xt_tiles[0][:, :4, :], xt_d[0, :, :4, :])
            nc.sync.dma_start(xt_tiles[0][:, 12:, :], xt_d[0, :, 12:, :])
            nc.sync.dma_start(w_sb[:, M_V], wqkv_d[M_V])
            ident_sb = consts.tile([128, 128], BF, name="ident_sb")
            nc.sync.dma_start(ident_sb, ident_d)
            nc.sync.dma_start(xt_tiles[1][:, :8, :], xt_d[1, :, :8, :])
            nc.sync.dma_start(xt_tiles[1][:, 8:, :], xt_d[1, :, 8:, :])
            woT_sb = consts.tile([128, 4, ND, 128], BF, name="woT_sb")
            for j in range(4):
                nc.sync.dma_start(woT_sb[:, j], woT_d[j])
            deferred = []   # DMAs to gate behind chunk-0's first matmul
            mask_sb = consts.tile([128, 512], BF, name="mask_sb")
            deferred.append(nc.gpsimd.dma_start(mask_sb, mask_d))
            onesc_sb = consts.tile([128, 128], BF, name="onesc_sb")
            deferred.append(nc.gpsimd.dma_start(onesc_sb, onesc_d))
            for i in range(1, NT):
                deferred.append(nc.gpsimd.dma_start(
                    cos_sb[:, ts(i, 512)], cos_d[:, ts(i, 512)]))
                deferred.append(nc.gpsimd.dma_start(
                    sin_sb[:, ts(i, 512)], sin_d[:, ts(i, 512)]))

            qt_sb = consts.tile([128, NH, S], BF, name="qt_sb")   # Q^T, rope'd
            kt_sb = consts.tile([128, S], BF, name="kt_sb")       # K^T, rope'd
            v_sb = consts.tile([128, ND, HD], BF, name="v_sb")    # V [tok, hd] blocks

            ag_in = [[dram.tile([128, 512], BF, name=f"agin{i}_{h}")
                      for h in range(NH)] for i in range(NT)]
            ag_out = [[dram.tile([512, 512], BF, name=f"agout{i}_{h}")
                       for h in range(NH)] for i in range(NT)]

            gates = {}  # (chunk, m) -> first matmul of that chain

            def proj_chunk(tc_i):
                xt_t = xt_tiles[tc_i]
                for m in range(6):  # k, v, 4 q heads
                    ps = psA.tile([128, 512], F32, tag="psA", name="ps_proj")
                    for d in range(ND):
                        mm = nc.tensor.matmul(
                            ps, lhsT=w_sb[:, m, d, :], rhs=xt_t[:, d, :],
                            start=(d == 0), stop=(d == ND - 1),
                        )
                        if d == 0:
                            gates[(tc_i, m)] = mm
                    if m != M_V:
                        # RoPE: out = raw*cos + swap(raw)*sin_signed
                        raw = work.tile([128, 512], BF, tag="rope_raw", name="raw")
                        nc.scalar.copy(raw, ps)
                        ps2 = psB.tile([128, 512], F32, tag="psB", name="ps_swap")
                        nc.tensor.matmul(ps2, lhsT=pswap_sb, rhs=raw,
                                         start=True, stop=True)
                        t1 = work.tile([128, 512], F32, tag="rope_t1", name="t1", bufs=2)
                        nc.vector.tensor_tensor(
                            t1, ps, cos_sb[:, ts(tc_i, 512)], mybir.AluOpType.mult)
                        t2 = work.tile([128, 512], F32, tag="rope_t2", name="t2", bufs=2)
                        nc.vector.tensor_tensor(
                            t2, ps2, sin_sb[:, ts(tc_i, 512)], mybir.AluOpType.mult)
                        dst = (kt_sb[:, ts(tc_i, 512)] if m == M_K
                               else qt_sb[:, m - M_Q0, ts(tc_i, 512)])
                        nc.vector.tensor_tensor(dst, t1, t2, mybir.AluOpType.add)
                    else:
                        # V^T chunk -> bf16 -> transpose to [tok, hd] blocks
                        vraw = work.tile([128, 512], BF, tag="rope_raw", name="vraw")
                        nc.scalar.copy(vraw, ps)
                        for j in range(4):
                            pst = psB.tile([128, 128], BF, tag="psB", name="ps_vT")
                            nc.tensor.transpose(pst, vraw[:, ts(j, 128)], ident_sb)
                            nc.vector.tensor_copy(v_sb[:, 4 * tc_i + j, :], pst)
                if tc_i == 0:
                    for dma in deferred:
                        add_dep_helper(dma.ins, gates[(0, M_K)].ins,
                                       reason="defer non-critical const DMA")

            def attn_chunk(qc):
                for h in range(NH):
                    ps_att = psB.tile([128, 512], F32, tag="psB", name="ps_att")
                    acc = work.tile([128, 512], BF, tag="ptsum", name="ptsum",
                                    bufs=2)
                    nkb = 4 * qc + 4
                    for kb in range(nkb):
                        r = kb - 4 * qc
                        o = max(r, 0) * 128   # first q column this kb can see
                        ps_s = psS.tile([128, 512], F32, tag="psS", name="ps_s")
                        nc.tensor.matmul(
                            ps_s[:, o:], lhsT=kt_sb[:, ts(kb, 128)],
                            rhs=qt_sb[:, h, 512 * qc + o:512 * (qc + 1)],
                            start=True, stop=True)
                        pt = work.tile([128, 512], BF, tag="pt", name="pt",
                                       bufs=6)
                        nc.scalar.activation(
                            pt[:, o:], ps_s[:, o:],
                            mybir.ActivationFunctionType.Exp, scale=SCALE)
                        if r >= 0:  # causal 0/1 mask on the hull, post-exp
                            nc.vector.tensor_tensor(
                                pt[:, o:], pt[:, o:],
                                mask_sb[:, :512 - o],
                                mybir.AluOpType.mult)
                        nc.tensor.matmul(
                            ps_att[:, o:], lhsT=v_sb[:, kb, :], rhs=pt[:, o:],
                            start=(kb == 0), stop=(kb == nkb - 1))
                        # softmax denominator: fold exp'd tiles on the DVE
                        # (kb==0 is full-width: either a full block or the
                        # r==0 diagonal whose mask zeroed the invalid part)
                        if kb == 0:
                            nc.vector.tensor_copy(acc, pt)
                        else:
                            nc.vector.tensor_tensor(
                                acc[:, o:], acc[:, o:], pt[:, o:],
                                mybir.AluOpType.add)
                    # ones[128,128] lhsT makes ps_den the partition-broadcast den
                    ps_den = psS.tile([128, 512], F32, tag="psS", name="ps_den")
                    nc.tensor.matmul(ps_den, lhsT=onesc_sb, rhs=acc,
                                     start=True, stop=True)
                    bden = work.tile([128, 512], F32, tag="bden", name="bden", bufs=2)
                    nc.vector.reciprocal_approx_fast(bden, ps_den)
                    att = work.tile([128, 512], BF, tag="att", name="att", bufs=2)
                    nc.vector.tensor_tensor(att, ps_att, bden,
                                            mybir.AluOpType.mult)
                    nc.scalar.dma_start(ag_in[qc][h], att)
                    nc.gpsimd.collective_compute(
                        "AllGather", mybir.AluOpType.bypass,
                        replica_groups=RG,
                        ins=[ag_in[qc][h][:].opt()],
                        outs=[ag_out[qc][h][:].opt()])

            def oproj_chunk(tc_i):
                rq = []
                reng = nc.sync if tc_i < 2 else nc.scalar
                for hq in range(NH):
                    r = io.tile([128, 4, 512], BF, tag="io128", name="oproj_r",
                                bufs=8)
                    reng.dma_start(
                        r, ag_out[tc_i][hq].rearrange("(o p) t -> p o t", p=128))
                    rq.append(r)
                for j in range(4):
                    ps_o = psA.tile([128, 512], F32, tag="psA", name="ps_o")
                    for c in range(ND):
                        nc.tensor.matmul(
                            ps_o, lhsT=woT_sb[:, j, c, :],
                            rhs=rq[c // 4][:, c % 4, :],
                            start=(c == 0), stop=(c == ND - 1))
                    o16 = work.tile([128, 512], BF, tag="o16", name="o16",
                                    bufs=2)
                    nc.vector.tensor_copy(o16, ps_o)
                    nc.sync.dma_start(out_d[ts(j, 128), ts(tc_i, 512)], o16)

            for i in range(NT):
                proj_chunk(i)
                attn_chunk(i)
            for i in range(NT):
                oproj_chunk(i)

    nc.compile()
    return nc


def make_in_maps(x, freqs_cos, freqs_sin, wq, wk, wv, wo):
    fc = np.asarray(freqs_cos, np.float32)
    fs = np.asarray(freqs_sin, np.float32)
    cos_exp = np.ascontiguousarray(np.repeat(fc.T, 2, axis=0)).astype(bf16)
    sgn = np.tile(np.array([-1.0, 1.0], np.float32), 64)[:, None]
    sin_sgn = np.ascontiguousarray(np.repeat(fs.T, 2, axis=0) * sgn).astype(bf16)
    mask01 = np.triu(np.ones((128, 512), np.float32), 0).astype(bf16)
    pswap = np.zeros((128, 128), np.float32)
    pswap[np.arange(128), np.arange(128) ^ 1] = 1.0
    pswap = pswap.astype(bf16)
    ident = np.eye(128, dtype=np.float32).astype(bf16)
    onesc = np.ones((128, 128), np.float32).astype(bf16)

    # x^T in chunk-major layout [chunk][p][o][t] so DMA lines are 4KB
    xt = []
    for b in range(B):
        t = np.ascontiguousarray(np.asarray(x[b], np.float32).T).astype(bf16)
        xt.append(np.ascontiguousarray(
            t.reshape(ND, 128, NT, 512).transpose(2, 1, 0, 3)))
    in_maps = []
    for core in range(8):
        b, g = divmod(core, 4)
        # m-order K, V, Q0..Q3 so attention deps clear earliest
        wqkvT = np.concatenate(
            [np.asarray(wk, np.float32)[128 * g:128 * (g + 1)].T,
             np.asarray(wv, np.float32)[128 * g:128 * (g + 1)].T,
             np.asarray(wq, np.float32)[512 * g:512 * (g + 1)].T], axis=1)
        # m-major SBUF-order blocks: [6][p 128][o*128+c 2048]
        wqkvT = np.ascontiguousarray(
            wqkvT.reshape(16, 128, 768).transpose(2, 1, 0)   # [768 m, 128 p, 16 o]
        )  # temp
        wqkvT = np.ascontiguousarray(np.stack(
            [wqkvT[128 * m:128 * (m + 1)].transpose(1, 2, 0).reshape(128, 2048)
             for m in range(6)])).reshape(6, 128, ND, 128)
        # oproj rhs block c = gathered head-quarter c//4, core c%4
        # = global head 4*(c%4) + c//4
        order = [0, 4, 8, 12, 1, 5, 9, 13, 2, 6, 10, 14, 3, 7, 11, 15]
        woT = np.asarray(wo, np.float32)[512 * g:512 * (g + 1), :].T
        woT = woT.reshape(16, 128, 512)[order].reshape(2048, 512)
        # j-major flat blocks: [j][p][o][c] so each DMA line is 4KB
        woT = np.ascontiguousarray(
            woT.reshape(16, 128, 4, 128).transpose(2, 1, 0, 3))
        in_maps.append({
            "xt": xt[b],
            "wqkvT": np.ascontiguousarray(wqkvT).astype(bf16),
            "woT": np.ascontiguousarray(woT).astype(bf16),
            "cose": cos_exp,
            "sins": sin_sgn,
            "mask01": mask01,
            "pswap": pswap,
            "ident": ident,
            "onesc": onesc,
        })
    return in_maps


_NC = None


def get_nc():
    global _NC
    if _NC is None:
        _NC = build_nc()
    return _NC


def assemble_out(results):
    out = np.zeros((B, S, D), np.float32)
    for core in range(8):
        b, g = divmod(core, 4)
        out[b, :, 512 * g:512 * (g + 1)] = np.asarray(results[core]["out"], np.float32).T
    return out


def kernel(x, freqs_cos, freqs_sin, wq, wk, wv, wo):
    import os
    os.environ.setdefault("BASS_NEVER_TRACE", "1")  # NTFF hook absent headless
    nc = get_nc()
    in_maps = make_in_maps(x, freqs_cos, freqs_sin, wq, wk, wv, wo)
    res = run_bass_kernel_spmd(nc, in_maps, core_ids=list(range(8)))
    return assemble_out(res.results)


# revision 7
# speedup vs baseline: 1.1085x; 1.1085x over previous
"""GQA attention (B=2,S=2048,D=2048,H=16,KV=4,HD=128) + RoPE on 8 TRN2 NeuronCores.

v1 fallback: 287917 ns measured. Baseline structure + denominator fold +
const-DMA reorder + K,V-first chains + bf16 rope tables + shrunk mask.

Sharding: core c -> (batch b=c//4, kv-group g=c%4). Each core projects
Q (4 heads), K/V (1 kv head) for its batch from a replicated x^T, applies
RoPE, runs causal flash attention (scores^T layout, no-max softmax --
|scores|<9 so fp32 exp is safe), AllGathers the per-head attention outputs
across the 4-core batch group, and computes a column slice of the output
projection (column-parallel wo).
"""
import numpy as np
import ml_dtypes

import concourse.bass as bass
import concourse.mybir as mybir
import concourse.tile as tile
from concourse import bacc
from concourse.bass import ts
from concourse.bass_utils import run_bass_kernel_spmd

BF = mybir.dt.bfloat16
F32 = mybir.dt.float32
bf16 = ml_dtypes.bfloat16

B, S, D = 2, 2048, 2048
H, KV, HD = 16, 4, 128
NT = 4          # 512-token chunks
ND = 16         # 128-wide D chunks
NH = 4          # heads per core
SCALE = 1.0 / np.sqrt(HD)
RG = [[0, 1, 2, 3], [4, 5, 6, 7]]
# projection chain order: m=0 K, m=1 V, m=2..5 Q heads 0..3
M_K, M_V, M_Q0 = 0, 1, 2


def build_nc():
    nc = bacc.Bacc("TRN2", target_bir_lowering=False, debug=False, num_devices=8)
    xt_d = nc.dram_tensor("xt", [D, S], BF, kind="ExternalInput").ap()
    wqkv_d = nc.dram_tensor("wqkvT", [6, 128, 2048], BF, kind="ExternalInput").ap()
    woT_d = nc.dram_tensor("woT", [D, 512], BF, kind="ExternalInput").ap()
    cos_d = nc.dram_tensor("cose", [128, S], BF, kind="ExternalInput").ap()
    sin_d = nc.dram_tensor("sins", [128, S], BF, kind="ExternalInput").ap()
    mask_d = nc.dram_tensor("mask01", [128, 512], BF, kind="ExternalInput").ap()
    pswap_d = nc.dram_tensor("pswap", [128, 128], BF, kind="ExternalInput").ap()
    ident_d = nc.dram_tensor("ident", [128, 128], BF, kind="ExternalInput").ap()
    onesc_d = nc.dram_tensor("onesc", [128, 128], BF, kind="ExternalInput").ap()
    out_d = nc.dram_tensor("out", [512, S], F32, kind="ExternalOutput").ap()

    xt_r = xt_d.rearrange("(o p) t -> p o t", p=128)      # [128, 16, 2048]
    woT_r = woT_d.rearrange("(o p) m -> p o m", p=128)    # [128, 16, 512]

    with tile.TileContext(nc) as tc:
        with (
            tc.tile_pool(name="consts", bufs=1) as consts,
            tc.tile_pool(name="io", bufs=2) as io,
            tc.tile_pool(name="work", bufs=3) as work,
            tc.tile_pool(name="psS", bufs=4, space="PSUM") as psS,
            tc.tile_pool(name="psA", bufs=2, space="PSUM") as psA,
            tc.tile_pool(name="psB", bufs=2, space="PSUM") as psB,
            tc.tile_pool(name="dram", bufs=1, space="DRAM") as dram,
        ):
            w_sb = consts.tile([128, 6, ND, 128], BF, name="w_sb")
            cos_sb = consts.tile([128, S], BF, name="cos_sb")
            sin_sb = consts.tile([128, S], BF, name="sin_sb")
            nc.gpsimd.dma_start(
                w_sb[:, M_K], wqkv_d[M_K].rearrange("p (o c) -> p o c", c=128))
            nc.gpsimd.dma_start(cos_sb[:, ts(0, 512)], cos_d[:, ts(0, 512)])
            nc.gpsimd.dma_start(sin_sb[:, ts(0, 512)], sin_d[:, ts(0, 512)])
            pswap_sb = consts.tile([128, 128], BF, name="pswap_sb")
            nc.gpsimd.dma_start(pswap_sb, pswap_d)
            nc.gpsimd.dma_start(
                w_sb[:, M_V], wqkv_d[M_V].rearrange("p (o c) -> p o c", c=128))
            ident_sb = consts.tile([128, 128], BF, name="ident_sb")
            nc.gpsimd.dma_start(ident_sb, ident_d)
            for m in range(M_Q0, 6):
                nc.gpsimd.dma_start(
                    w_sb[:, m], wqkv_d[m].rearrange("p (o c) -> p o c", c=128))
            mask_sb = consts.tile([128, 512], BF, name="mask_sb")
            nc.gpsimd.dma_start(mask_sb, mask_d)
            onesc_sb = consts.tile([128, 128], BF, name="onesc_sb")
            nc.gpsimd.dma_start(onesc_sb, onesc_d)
            for i in range(1, NT):
                nc.gpsimd.dma_start(cos_sb[:, ts(i, 512)], cos_d[:, ts(i, 512)])
                nc.gpsimd.dma_start(sin_sb[:, ts(i, 512)], sin_d[:, ts(i, 512)])
            woT_sb = consts.tile([128, ND, 512], BF, name="woT_sb")
            nc.gpsimd.dma_start(woT_sb, woT_r)

            qt_sb = consts.tile([128, NH, S], BF, name="qt_sb")   # Q^T, rope'd
            kt_sb = consts.tile([128, S], BF, name="kt_sb")       # K^T, rope'd
            v_sb = consts.tile([128, ND, HD], BF, name="v_sb")    # V [tok, hd] blocks

            ag_in = [[dram.tile([256, 512], BF, name=f"agin{i}_{p}")
                      for p in range(2)] for i in range(NT)]
            ag_out = [[dram.tile([1024, 512], BF, name=f"agout{i}_{p}")
                       for p in range(2)] for i in range(NT)]

            def proj_chunk(tc_i):
                xt_t = io.tile([128, ND, 512], BF, tag="io512", name="xt_t")
                qengs = ([nc.sync, nc.scalar, nc.scalar, nc.sync]
                         if tc_i == 0 else [nc.sync] * 4)
                for q in range(4):
                    qengs[q].dma_start(xt_t[:, 4 * q:4 * (q + 1), :],
                                       xt_r[:, 4 * q:4 * (q + 1), ts(tc_i, 512)])
                for m in range(6):  # k, v, 4 q heads
                    ps = psA.tile([128, 512], F32, tag="psA", name="ps_proj")
                    for d in range(ND):
                        nc.tensor.matmul(
                            ps, lhsT=w_sb[:, m, d, :], rhs=xt_t[:, d, :],
                            start=(d == 0), stop=(d == ND - 1),
                        )
                    if m != M_V:
                        raw = work.tile([128, 512], BF, tag="rope_raw", name="raw")
                        nc.scalar.copy(raw, ps)
                        ps2 = psB.tile([128, 512], F32, tag="psB", name="ps_swap")
                        nc.tensor.matmul(ps2, lhsT=pswap_sb, rhs=raw,
                                         start=True, stop=True)
                        t1 = work.tile([128, 512], F32, tag="rope_t1", name="t1")
                        nc.vector.tensor_tensor(
                            t1, ps, cos_sb[:, ts(tc_i, 512)], mybir.AluOpType.mult)
                        t2 = work.tile([128, 512], F32, tag="rope_t2", name="t2")
                        nc.vector.tensor_tensor(
                            t2, ps2, sin_sb[:, ts(tc_i, 512)], mybir.AluOpType.mult)
                        dst = (kt_sb[:, ts(tc_i, 512)] if m == M_K
                               else qt_sb[:, m - M_Q0, ts(tc_i, 512)])
                        nc.vector.tensor_tensor(dst, t1, t2, mybir.AluOpType.add)
                    else:
                        vraw = work.tile([128, 512], BF, tag="rope_raw", name="vraw")
                        nc.scalar.copy(vraw, ps)
                        for j in range(4):
                            pst = psB.tile([128, 128], BF, tag="psB", name="ps_vT")
                            nc.tensor.transpose(pst, vraw[:, ts(j, 128)], ident_sb)
                            nc.vector.tensor_copy(v_sb[:, 4 * tc_i + j, :], pst)

            def attn_chunk(qc):
                for h in range(NH):
                    ps_att = psB.tile([128, 512], F32, tag="psB", name="ps_att")
                    acc = work.tile([128, 512], BF, tag="ptsum", name="ptsum",
                                    bufs=2)
                    nkb = 4 * qc + 4
                    for kb in range(nkb):
                        r = kb - 4 * qc
                        o = max(r, 0) * 128
                        ps_s = psS.tile([128, 512], F32, tag="psS", name="ps_s")
                        nc.tensor.matmul(
                            ps_s[:, o:], lhsT=kt_sb[:, ts(kb, 128)],
                            rhs=qt_sb[:, h, 512 * qc + o:512 * (qc + 1)],
                            start=True, stop=True)
                        pt = work.tile([128, 512], BF, tag="pt", name="pt",
                                       bufs=6)
                        nc.scalar.activation(
                            pt[:, o:], ps_s[:, o:],
                            mybir.ActivationFunctionType.Exp, scale=SCALE)
                        if r >= 0:
                            nc.vector.tensor_tensor(
                                pt[:, o:], pt[:, o:],
                                mask_sb[:, :512 - o],
                                mybir.AluOpType.mult)
                        nc.tensor.matmul(
                            ps_att[:, o:], lhsT=v_sb[:, kb, :], rhs=pt[:, o:],
                            start=(kb == 0), stop=(kb == nkb - 1))
                        if kb == 0:
                            nc.vector.tensor_copy(acc, pt)
                        else:
                            nc.vector.tensor_tensor(
                                acc[:, o:], acc[:, o:], pt[:, o:],
                                mybir.AluOpType.add)
                    ps_den = psS.tile([128, 512], F32, tag="psS", name="ps_den")
                    nc.tensor.matmul(ps_den, lhsT=onesc_sb, rhs=acc,
                                     start=True, stop=True)
                    bden = work.tile([128, 512], F32, tag="bden", name="bden")
                    nc.vector.reciprocal_approx_fast(bden, ps_den)
                    att = work.tile([128, 512], BF, tag="att", name="att")
                    nc.vector.tensor_tensor(att, ps_att, bden,
                                            mybir.AluOpType.mult)
                    nc.scalar.dma_start(ag_in[qc][h // 2][ts(h % 2, 128), :], att)
                    if h % 2 == 1:
                        nc.gpsimd.collective_compute(
                            "AllGather", mybir.AluOpType.bypass,
                            replica_groups=RG,
                            ins=[ag_in[qc][h // 2][:].opt()],
                            outs=[ag_out[qc][h // 2][:].opt()])

            def oproj_chunk(tc_i):
                rhs = io.tile([128, ND, 512], BF, tag="io512", name="oproj_rhs")
                nc.sync.dma_start(
                    rhs[:, :8, :],
                    ag_out[tc_i][0].rearrange("(o p) t -> p o t", p=128))
                nc.sync.dma_start(
                    rhs[:, 8:, :],
                    ag_out[tc_i][1].rearrange("(o p) t -> p o t", p=128))
                for j in range(4):
                    ps_o = psA.tile([128, 512], F32, tag="psA", name="ps_o")
                    for c in range(ND):
                        nc.tensor.matmul(
                            ps_o, lhsT=woT_sb[:, c, ts(j, 128)], rhs=rhs[:, c, :],
                            start=(c == 0), stop=(c == ND - 1))
                    o32 = work.tile([128, 512], F32, tag="o32", name="o32")
                    nc.vector.tensor_copy(o32, ps_o)
                    nc.sync.dma_start(out_d[ts(j, 128), ts(tc_i, 512)], o32)

            for i in range(NT):
                proj_chunk(i)
                attn_chunk(i)
            for i in range(NT):
                oproj_chunk(i)

    nc.compile()
    return nc


def make_in_maps(x, freqs_cos, freqs_sin, wq, wk, wv, wo):
    fc = np.asarray(freqs_cos, np.float32)
    fs = np.asarray(freqs_sin, np.float32)
    cos_exp = np.ascontiguousarray(np.repeat(fc.T, 2, axis=0)).astype(bf16)
    sgn = np.tile(np.array([-1.0, 1.0], np.float32), 64)[:, None]
    sin_sgn = np.ascontiguousarray(np.repeat(fs.T, 2, axis=0) * sgn).astype(bf16)
    mask01 = np.triu(np.ones((128, 512), np.float32), 0).astype(bf16)
    pswap = np.zeros((128, 128), np.float32)
    pswap[np.arange(128), np.arange(128) ^ 1] = 1.0
    pswap = pswap.astype(bf16)
    ident = np.eye(128, dtype=np.float32).astype(bf16)
    onesc = np.ones((128, 128), np.float32).astype(bf16)

    xt = [np.ascontiguousarray(np.asarray(x[b], np.float32).T).astype(bf16)
          for b in range(B)]
    in_maps = []
    for core in range(8):
        b, g = divmod(core, 4)
        wqkvT = np.concatenate(
            [np.asarray(wk, np.float32)[128 * g:128 * (g + 1)].T,
             np.asarray(wv, np.float32)[128 * g:128 * (g + 1)].T,
             np.asarray(wq, np.float32)[512 * g:512 * (g + 1)].T], axis=1)
        wqkvT = np.ascontiguousarray(
            wqkvT.reshape(16, 128, 768).transpose(2, 1, 0)
        )
        wqkvT = np.ascontiguousarray(np.stack(
            [wqkvT[128 * m:128 * (m + 1)].transpose(1, 2, 0).reshape(128, 2048)
             for m in range(6)]))
        order = [0, 1, 4, 5, 8, 9, 12, 13, 2, 3, 6, 7, 10, 11, 14, 15]
        woT = np.asarray(wo, np.float32)[512 * g:512 * (g + 1), :].T
        woT = woT.reshape(16, 128, 512)[order].reshape(2048, 512)
        in_maps.append({
            "xt": xt[b],
            "wqkvT": np.ascontiguousarray(wqkvT).astype(bf16),
            "woT": np.ascontiguousarray(woT).astype(bf16),
            "cose": cos_exp,
            "sins": sin_sgn,
            "mask01": mask01,
            "pswap": pswap,
            "ident": ident,
            "onesc": onesc,
        })
    return in_maps


_NC = None


def get_nc():
    global _NC
    if _NC is None:
        _NC = build_nc()
    return _NC


def assemble_out(results):
    out = np.zeros((B, S, D), np.float32)
    for core in range(8):
        b, g = divmod(core, 4)
        out[b, :, 512 * g:512 * (g + 1)] = results[core]["out"].T
    return out


def kernel(x, freqs_cos, freqs_sin, wq, wk, wv, wo):
    import os
    os.environ.setdefault("BASS_NEVER_TRACE", "1")  # NTFF hook absent headless
    nc = get_nc()
    in_maps = make_in_maps(x, freqs_cos, freqs_sin, wq, wk, wv, wo)
    res = run_bass_kernel_spmd(nc, in_maps, core_ids=list(range(8)))
    return assemble_out(res.results)


# revision 8
# speedup vs baseline: 1.1675x; 1.0532x over previous
"""GQA attention (B=2,S=2048,D=2048,H=16,KV=4,HD=128) + RoPE on 8 TRN2 NeuronCores.

v1 fallback: 287917 ns measured. Baseline structure + denominator fold +
const-DMA reorder + K,V-first chains + bf16 rope tables + shrunk mask.

Sharding: core c -> (batch b=c//4, kv-group g=c%4). Each core projects
Q (4 heads), K/V (1 kv head) for its batch from a replicated x^T, applies
RoPE, runs causal flash attention (scores^T layout, no-max softmax --
|scores|<9 so fp32 exp is safe), AllGathers the per-head attention outputs
across the 4-core batch group, and computes a column slice of the output
projection (column-parallel wo).
"""
import numpy as np
import ml_dtypes

import concourse.bass as bass
import concourse.mybir as mybir
import concourse.tile as tile
from concourse import bacc
from concourse.bass import ts
from concourse.bass_utils import run_bass_kernel_spmd

BF = mybir.dt.bfloat16
F32 = mybir.dt.float32
bf16 = ml_dtypes.bfloat16

B, S, D = 2, 2048, 2048
H, KV, HD = 16, 4, 128
NT = 4          # 512-token chunks
ND = 16         # 128-wide D chunks
NH = 4          # heads per core
SCALE = 1.0 / np.sqrt(HD)
RG = [[0, 1, 2, 3], [4, 5, 6, 7]]
# projection chain order: m=0 K, m=1 V, m=2..5 Q heads 0..3
M_K, M_V, M_Q0 = 0, 1, 2


def build_nc():
    nc = bacc.Bacc("TRN2", target_bir_lowering=False, debug=False, num_devices=8)
    xt_d = nc.dram_tensor("xt", [NT, 128, ND, 512], BF, kind="ExternalInput").ap()
    wqkv_d = nc.dram_tensor("wqkvT", [6, 128, 2048], BF, kind="ExternalInput").ap()
    woT_d = nc.dram_tensor("woT", [D, 512], BF, kind="ExternalInput").ap()
    cos_d = nc.dram_tensor("cose", [128, S], BF, kind="ExternalInput").ap()
    sin_d = nc.dram_tensor("sins", [128, S], BF, kind="ExternalInput").ap()
    mask_d = nc.dram_tensor("mask01", [128, 512], BF, kind="ExternalInput").ap()
    pswap_d = nc.dram_tensor("pswap", [128, 128], BF, kind="ExternalInput").ap()
    ident_d = nc.dram_tensor("ident", [128, 128], BF, kind="ExternalInput").ap()
    onesc_d = nc.dram_tensor("onesc", [128, 128], BF, kind="ExternalInput").ap()
    out_d = nc.dram_tensor("out", [512, S], BF, kind="ExternalOutput").ap()

    woT_r = woT_d.rearrange("(o p) m -> p o m", p=128)    # [128, 16, 512]

    with tile.TileContext(nc) as tc:
        with (
            tc.tile_pool(name="consts", bufs=1) as consts,
            tc.tile_pool(name="io", bufs=2) as io,
            tc.tile_pool(name="work", bufs=3) as work,
            tc.tile_pool(name="psS", bufs=4, space="PSUM") as psS,
            tc.tile_pool(name="psA", bufs=2, space="PSUM") as psA,
            tc.tile_pool(name="psB", bufs=2, space="PSUM") as psB,
            tc.tile_pool(name="dram", bufs=1, space="DRAM") as dram,
        ):
            w_sb = consts.tile([128, 6, ND, 128], BF, name="w_sb")
            cos_sb = consts.tile([128, S], BF, name="cos_sb")
            sin_sb = consts.tile([128, S], BF, name="sin_sb")
            nc.gpsimd.dma_start(
                w_sb[:, M_K], wqkv_d[M_K].rearrange("p (o c) -> p o c", c=128))
            nc.gpsimd.dma_start(cos_sb[:, ts(0, 512)], cos_d[:, ts(0, 512)])
            nc.gpsimd.dma_start(sin_sb[:, ts(0, 512)], sin_d[:, ts(0, 512)])
            pswap_sb = consts.tile([128, 128], BF, name="pswap_sb")
            nc.gpsimd.dma_start(pswap_sb, pswap_d)
            nc.gpsimd.dma_start(
                w_sb[:, M_V], wqkv_d[M_V].rearrange("p (o c) -> p o c", c=128))
            ident_sb = consts.tile([128, 128], BF, name="ident_sb")
            nc.gpsimd.dma_start(ident_sb, ident_d)
            for m in range(M_Q0, 6):
                nc.gpsimd.dma_start(
                    w_sb[:, m], wqkv_d[m].rearrange("p (o c) -> p o c", c=128))
            mask_sb = consts.tile([128, 512], BF, name="mask_sb")
            nc.gpsimd.dma_start(mask_sb, mask_d)
            onesc_sb = consts.tile([128, 128], BF, name="onesc_sb")
            nc.gpsimd.dma_start(onesc_sb, onesc_d)
            for i in range(1, NT):
                nc.gpsimd.dma_start(cos_sb[:, ts(i, 512)], cos_d[:, ts(i, 512)])
                nc.gpsimd.dma_start(sin_sb[:, ts(i, 512)], sin_d[:, ts(i, 512)])
            woT_sb = consts.tile([128, ND, 512], BF, name="woT_sb")
            nc.gpsimd.dma_start(woT_sb, woT_r)

            qt_sb = consts.tile([128, NH, S], BF, name="qt_sb")   # Q^T, rope'd
            kt_sb = consts.tile([128, S], BF, name="kt_sb")       # K^T, rope'd
            v_sb = consts.tile([128, ND, HD], BF, name="v_sb")    # V [tok, hd] blocks

            ag_in = [[dram.tile([256, 512], BF, name=f"agin{i}_{p}")
                      for p in range(2)] for i in range(NT)]
            ag_out = [[dram.tile([1024, 512], BF, name=f"agout{i}_{p}")
                       for p in range(2)] for i in range(NT)]

            def proj_chunk(tc_i):
                xt_t = io.tile([128, ND, 512], BF, tag="io512", name="xt_t")
                qengs = ([nc.sync, nc.scalar, nc.scalar, nc.sync]
                         if tc_i == 0 else [nc.sync] * 4)
                for q in range(4):
                    qengs[q].dma_start(xt_t[:, 4 * q:4 * (q + 1), :],
                                       xt_d[tc_i, :, 4 * q:4 * (q + 1), :])
                for m in range(6):  # k, v, 4 q heads
                    ps = psA.tile([128, 512], F32, tag="psA", name="ps_proj")
                    for d in range(ND):
                        nc.tensor.matmul(
                            ps, lhsT=w_sb[:, m, d, :], rhs=xt_t[:, d, :],
                            start=(d == 0), stop=(d == ND - 1),
                        )
                    if m != M_V:
                        raw = work.tile([128, 512], BF, tag="rope_raw", name="raw")
                        nc.scalar.copy(raw, ps)
                        ps2 = psB.tile([128, 512], F32, tag="psB", name="ps_swap")
                        nc.tensor.matmul(ps2, lhsT=pswap_sb, rhs=raw,
                                         start=True, stop=True)
                        t1 = work.tile([128, 512], F32, tag="rope_t1", name="t1")
                        nc.vector.tensor_tensor(
                            t1, ps, cos_sb[:, ts(tc_i, 512)], mybir.AluOpType.mult)
                        t2 = work.tile([128, 512], F32, tag="rope_t2", name="t2")
                        nc.vector.tensor_tensor(
                            t2, ps2, sin_sb[:, ts(tc_i, 512)], mybir.AluOpType.mult)
                        dst = (kt_sb[:, ts(tc_i, 512)] if m == M_K
                               else qt_sb[:, m - M_Q0, ts(tc_i, 512)])
                        nc.vector.tensor_tensor(dst, t1, t2, mybir.AluOpType.add)
                    else:
                        vraw = work.tile([128, 512], BF, tag="rope_raw", name="vraw")
                        nc.scalar.copy(vraw, ps)
                        for j in range(4):
                            pst = psB.tile([128, 128], BF, tag="psB", name="ps_vT")
                            nc.tensor.transpose(pst, vraw[:, ts(j, 128)], ident_sb)
                            nc.vector.tensor_copy(v_sb[:, 4 * tc_i + j, :], pst)

            def attn_chunk(qc):
                for h in range(NH):
                    ps_att = psB.tile([128, 512], F32, tag="psB", name="ps_att")
                    acc = work.tile([128, 512], BF, tag="ptsum", name="ptsum",
                                    bufs=2)
                    nkb = 4 * qc + 4
                    for kb in range(nkb):
                        r = kb - 4 * qc
                        o = max(r, 0) * 128
                        ps_s = psS.tile([128, 512], F32, tag="psS", name="ps_s")
                        nc.tensor.matmul(
                            ps_s[:, o:], lhsT=kt_sb[:, ts(kb, 128)],
                            rhs=qt_sb[:, h, 512 * qc + o:512 * (qc + 1)],
                            start=True, stop=True)
                        pt = work.tile([128, 512], BF, tag="pt", name="pt",
                                       bufs=6)
                        nc.scalar.activation(
                            pt[:, o:], ps_s[:, o:],
                            mybir.ActivationFunctionType.Exp, scale=SCALE)
                        if r >= 0:
                            nc.vector.tensor_tensor(
                                pt[:, o:], pt[:, o:],
                                mask_sb[:, :512 - o],
                                mybir.AluOpType.mult)
                        nc.tensor.matmul(
                            ps_att[:, o:], lhsT=v_sb[:, kb, :], rhs=pt[:, o:],
                            start=(kb == 0), stop=(kb == nkb - 1))
                        if kb == 0:
                            nc.vector.tensor_copy(acc, pt)
                        else:
                            nc.vector.tensor_tensor(
                                acc[:, o:], acc[:, o:], pt[:, o:],
                                mybir.AluOpType.add)
                    ps_den = psS.tile([128, 512], F32, tag="psS", name="ps_den")
                    nc.tensor.matmul(ps_den, lhsT=onesc_sb, rhs=acc,
                                     start=True, stop=True)
                    bden = work.tile([128, 512], F32, tag="bden", name="bden")
                    nc.vector.reciprocal_approx_fast(bden, ps_den)
                    att = work.tile([128, 512], BF, tag="att", name="att")
                    nc.vector.tensor_tensor(att, ps_att, bden,
                                            mybir.AluOpType.mult)
                    nc.scalar.dma_start(ag_in[qc][h // 2][ts(h % 2, 128), :], att)
                    if h % 2 == 1:
                        nc.gpsimd.collective_compute(
                            "AllGather", mybir.AluOpType.bypass,
                            replica_groups=RG,
                            ins=[ag_in[qc][h // 2][:].opt()],
                            outs=[ag_out[qc][h // 2][:].opt()])

            def oproj_chunk(tc_i):
                # two rhs tiles -> the j-chains' first 8 steps depend only on
                # the first pair-gather, so they overlap the last attention
                # blocks instead of waiting for both gathers
                rhsA = io.tile([128, 8, 512], BF, tag="io256", name="oproj_rA",
                               bufs=4)
                nc.sync.dma_start(
                    rhsA, ag_out[tc_i][0].rearrange("(o p) t -> p o t", p=128))
                rhsB = io.tile([128, 8, 512], BF, tag="io256", name="oproj_rB",
                               bufs=4)
                nc.sync.dma_start(
                    rhsB, ag_out[tc_i][1].rearrange("(o p) t -> p o t", p=128))
                for j in range(4):
                    ps_o = psA.tile([128, 512], F32, tag="psA", name="ps_o")
                    for c in range(ND):
                        rhs_c = rhsA[:, c, :] if c < 8 else rhsB[:, c - 8, :]
                        nc.tensor.matmul(
                            ps_o, lhsT=woT_sb[:, c, ts(j, 128)], rhs=rhs_c,
                            start=(c == 0), stop=(c == ND - 1))
                    o16 = work.tile([128, 512], BF, tag="o16", name="o16")
                    nc.vector.tensor_copy(o16, ps_o)
                    nc.sync.dma_start(out_d[ts(j, 128), ts(tc_i, 512)], o16)

            for i in range(NT):
                proj_chunk(i)
                attn_chunk(i)
            for i in range(NT):
                oproj_chunk(i)

    nc.compile()
    return nc


def make_in_maps(x, freqs_cos, freqs_sin, wq, wk, wv, wo):
    fc = np.asarray(freqs_cos, np.float32)
    fs = np.asarray(freqs_sin, np.float32)
    cos_exp = np.ascontiguousarray(np.repeat(fc.T, 2, axis=0)).astype(bf16)
    sgn = np.tile(np.array([-1.0, 1.0], np.float32), 64)[:, None]
    sin_sgn = np.ascontiguousarray(np.repeat(fs.T, 2, axis=0) * sgn).astype(bf16)
    mask01 = np.triu(np.ones((128, 512), np.float32), 0).astype(bf16)
    pswap = np.zeros((128, 128), np.float32)
    pswap[np.arange(128), np.arange(128) ^ 1] = 1.0
    pswap = pswap.astype(bf16)
    ident = np.eye(128, dtype=np.float32).astype(bf16)
    onesc = np.ones((128, 128), np.float32).astype(bf16)

    # x^T in chunk-major layout [chunk][p][o][t] so DMA lines are 4KB
    xt = []
    for b in range(B):
        t = np.ascontiguousarray(np.asarray(x[b], np.float32).T).astype(bf16)
        xt.append(np.ascontiguousarray(
            t.reshape(ND, 128, NT, 512).transpose(2, 1, 0, 3)))
    in_maps = []
    for core in range(8):
        b, g = divmod(core, 4)
        wqkvT = np.concatenate(
            [np.asarray(wk, np.float32)[128 * g:128 * (g + 1)].T,
             np.asarray(wv, np.float32)[128 * g:128 * (g + 1)].T,
             np.asarray(wq, np.float32)[512 * g:512 * (g + 1)].T], axis=1)
        wqkvT = np.ascontiguousarray(
            wqkvT.reshape(16, 128, 768).transpose(2, 1, 0)
        )
        wqkvT = np.ascontiguousarray(np.stack(
            [wqkvT[128 * m:128 * (m + 1)].transpose(1, 2, 0).reshape(128, 2048)
             for m in range(6)]))
        order = [0, 1, 4, 5, 8, 9, 12, 13, 2, 3, 6, 7, 10, 11, 14, 15]
        woT = np.asarray(wo, np.float32)[512 * g:512 * (g + 1), :].T
        woT = woT.reshape(16, 128, 512)[order].reshape(2048, 512)
        in_maps.append({
            "xt": xt[b],
            "wqkvT": np.ascontiguousarray(wqkvT).astype(bf16),
            "woT": np.ascontiguousarray(woT).astype(bf16),
            "cose": cos_exp,
            "sins": sin_sgn,
            "mask01": mask01,
            "pswap": pswap,
            "ident": ident,
            "onesc": onesc,
        })
    return in_maps


_NC = None


def get_nc():
    global _NC
    if _NC is None:
        _NC = build_nc()
    return _NC


def assemble_out(results):
    out = np.zeros((B, S, D), np.float32)
    for core in range(8):
        b, g = divmod(core, 4)
        out[b, :, 512 * g:512 * (g + 1)] = np.asarray(results[core]["out"], np.float32).T
    return out


def kernel(x, freqs_cos, freqs_sin, wq, wk, wv, wo):
    import os
    os.environ.setdefault("BASS_NEVER_TRACE", "1")  # NTFF hook absent headless
    nc = get_nc()
    in_maps = make_in_maps(x, freqs_cos, freqs_sin, wq, wk, wv, wo)
    res = run_bass_kernel_spmd(nc, in_maps, core_ids=list(range(8)))
    return assemble_out(res.results)


# revision 9
# speedup vs baseline: 1.1741x; 1.0056x over previous
"""GQA attention (B=2,S=2048,D=2048,H=16,KV=4,HD=128) + RoPE on 8 TRN2 NeuronCores.

v1 fallback: 287917 ns measured. Baseline structure + denominator fold +
const-DMA reorder + K,V-first chains + bf16 rope tables + shrunk mask.

Sharding: core c -> (batch b=c//4, kv-group g=c%4). Each core projects
Q (4 heads), K/V (1 kv head) for its batch from a replicated x^T, applies
RoPE, runs causal flash attention (scores^T layout, no-max softmax --
|scores|<9 so fp32 exp is safe), AllGathers the per-head attention outputs
across the 4-core batch group, and computes a column slice of the output
projection (column-parallel wo).
"""
import numpy as np
import ml_dtypes

import concourse.bass as bass
import concourse.mybir as mybir
import concourse.tile as tile
from concourse import bacc
from concourse.bass import ts
from concourse.bass_utils import run_bass_kernel_spmd

BF = mybir.dt.bfloat16
F32 = mybir.dt.float32
bf16 = ml_dtypes.bfloat16

B, S, D = 2, 2048, 2048
H, KV, HD = 16, 4, 128
NT = 4          # 512-token chunks
ND = 16         # 128-wide D chunks
NH = 4          # heads per core
SCALE = 1.0 / np.sqrt(HD)
RG = [[0, 1, 2, 3], [4, 5, 6, 7]]
# projection chain order: m=0 K, m=1 V, m=2..5 Q heads 0..3
M_K, M_V, M_Q0 = 0, 1, 2


def build_nc():
    nc = bacc.Bacc("TRN2", target_bir_lowering=False, debug=False, num_devices=8)
    xt_d = nc.dram_tensor("xt", [NT, 128, ND, 512], BF, kind="ExternalInput").ap()
    wqkv_d = nc.dram_tensor("wqkvT", [6, 128, 2048], BF, kind="ExternalInput").ap()
    woT_d = nc.dram_tensor("woT", [D, 512], BF, kind="ExternalInput").ap()
    cos_d = nc.dram_tensor("cose", [128, S], BF, kind="ExternalInput").ap()
    sin_d = nc.dram_tensor("sins", [128, S], BF, kind="ExternalInput").ap()
    mask_d = nc.dram_tensor("mask01", [128, 512], BF, kind="ExternalInput").ap()
    pswap_d = nc.dram_tensor("pswap", [128, 128], BF, kind="ExternalInput").ap()
    ident_d = nc.dram_tensor("ident", [128, 128], BF, kind="ExternalInput").ap()
    onesc_d = nc.dram_tensor("onesc", [128, 128], BF, kind="ExternalInput").ap()
    out_d = nc.dram_tensor("out", [512, S], BF, kind="ExternalOutput").ap()

    woT_r = woT_d.rearrange("(o p) m -> p o m", p=128)    # [128, 16, 512]

    with tile.TileContext(nc) as tc:
        with (
            tc.tile_pool(name="consts", bufs=1) as consts,
            tc.tile_pool(name="io", bufs=2) as io,
            tc.tile_pool(name="work", bufs=3) as work,
            tc.tile_pool(name="psS", bufs=4, space="PSUM") as psS,
            tc.tile_pool(name="psA", bufs=2, space="PSUM") as psA,
            tc.tile_pool(name="psB", bufs=2, space="PSUM") as psB,
            tc.tile_pool(name="dram", bufs=1, space="DRAM") as dram,
        ):
            w_sb = consts.tile([128, 6, ND, 128], BF, name="w_sb")
            cos_sb = consts.tile([128, S], BF, name="cos_sb")
            sin_sb = consts.tile([128, S], BF, name="sin_sb")
            wk_r = wqkv_d[M_K].rearrange("p (o c) -> p o c", c=128)
            nc.gpsimd.dma_start(w_sb[:, M_K, :8], wk_r[:, :8])
            nc.gpsimd.dma_start(w_sb[:, M_K, 8:], wk_r[:, 8:])
            nc.gpsimd.dma_start(cos_sb[:, ts(0, 512)], cos_d[:, ts(0, 512)])
            nc.gpsimd.dma_start(sin_sb[:, ts(0, 512)], sin_d[:, ts(0, 512)])
            pswap_sb = consts.tile([128, 128], BF, name="pswap_sb")
            nc.gpsimd.dma_start(pswap_sb, pswap_d)
            nc.gpsimd.dma_start(
                w_sb[:, M_V], wqkv_d[M_V].rearrange("p (o c) -> p o c", c=128))
            ident_sb = consts.tile([128, 128], BF, name="ident_sb")
            nc.gpsimd.dma_start(ident_sb, ident_d)
            for m in (M_Q0, M_Q0 + 1):
                nc.gpsimd.dma_start(
                    w_sb[:, m], wqkv_d[m].rearrange("p (o c) -> p o c", c=128))
            mask_sb = consts.tile([128, 512], BF, name="mask_sb")
            nc.gpsimd.dma_start(mask_sb, mask_d)
            onesc_sb = consts.tile([128, 128], BF, name="onesc_sb")
            nc.gpsimd.dma_start(onesc_sb, onesc_d)
            for i in range(1, NT):
                nc.gpsimd.dma_start(cos_sb[:, ts(i, 512)], cos_d[:, ts(i, 512)])
                nc.gpsimd.dma_start(sin_sb[:, ts(i, 512)], sin_d[:, ts(i, 512)])
            woT_sb = consts.tile([128, ND, 512], BF, name="woT_sb")
            nc.gpsimd.dma_start(woT_sb, woT_r)

            qt_sb = consts.tile([128, NH, S], BF, name="qt_sb")   # Q^T, rope'd
            kt_sb = consts.tile([128, S], BF, name="kt_sb")       # K^T, rope'd
            v_sb = consts.tile([128, ND, HD], BF, name="v_sb")    # V [tok, hd] blocks

            ag_in = [[dram.tile([256, 512], BF, name=f"agin{i}_{p}")
                      for p in range(2)] for i in range(NT)]
            ag_out = [[dram.tile([1024, 512], BF, name=f"agout{i}_{p}")
                       for p in range(2)] for i in range(NT)]

            def proj_chunk(tc_i):
                xt_t = io.tile([128, ND, 512], BF, tag="io512", name="xt_t")
                qengs = ([nc.sync, nc.scalar, nc.scalar, nc.sync]
                         if tc_i == 0 else [nc.sync] * 4)
                for q in range(4):
                    if tc_i == 0 and q == 0:   # finest split for the first gate
                        nc.sync.dma_start(xt_t[:, :2, :], xt_d[0, :, :2, :])
                        nc.sync.dma_start(xt_t[:, 2:4, :], xt_d[0, :, 2:4, :])
                        continue
                    qengs[q].dma_start(xt_t[:, 4 * q:4 * (q + 1), :],
                                       xt_d[tc_i, :, 4 * q:4 * (q + 1), :])
                if tc_i == 0:
                    for m in (M_Q0 + 2, M_Q0 + 3):
                        nc.scalar.dma_start(
                            w_sb[:, m],
                            wqkv_d[m].rearrange("p (o c) -> p o c", c=128))
                for m in range(6):  # k, v, 4 q heads
                    ps = psA.tile([128, 512], F32, tag="psA", name="ps_proj")
                    for d in range(ND):
                        nc.tensor.matmul(
                            ps, lhsT=w_sb[:, m, d, :], rhs=xt_t[:, d, :],
                            start=(d == 0), stop=(d == ND - 1),
                        )
                    if m != M_V:
                        raw = work.tile([128, 512], BF, tag="rope_raw", name="raw")
                        nc.scalar.copy(raw, ps)
                        ps2 = psB.tile([128, 512], F32, tag="psB", name="ps_swap")
                        nc.tensor.matmul(ps2, lhsT=pswap_sb, rhs=raw,
                                         start=True, stop=True)
                        t1 = work.tile([128, 512], F32, tag="rope_t1", name="t1")
                        nc.vector.tensor_tensor(
                            t1, ps, cos_sb[:, ts(tc_i, 512)], mybir.AluOpType.mult)
                        t2 = work.tile([128, 512], F32, tag="rope_t2", name="t2")
                        nc.vector.tensor_tensor(
                            t2, ps2, sin_sb[:, ts(tc_i, 512)], mybir.AluOpType.mult)
                        dst = (kt_sb[:, ts(tc_i, 512)] if m == M_K
                               else qt_sb[:, m - M_Q0, ts(tc_i, 512)])
                        nc.vector.tensor_tensor(dst, t1, t2, mybir.AluOpType.add)
                    else:
                        vraw = work.tile([128, 512], BF, tag="rope_raw", name="vraw")
                        nc.scalar.copy(vraw, ps)
                        for j in range(4):
                            pst = psB.tile([128, 128], BF, tag="psB", name="ps_vT")
                            nc.tensor.transpose(pst, vraw[:, ts(j, 128)], ident_sb)
                            nc.vector.tensor_copy(v_sb[:, 4 * tc_i + j, :], pst)

            def attn_chunk(qc):
                for h in range(NH):
                    ps_att = psB.tile([128, 512], F32, tag="psB", name="ps_att")
                    acc = work.tile([128, 512], BF, tag="ptsum", name="ptsum",
                                    bufs=2)
                    nkb = 4 * qc + 4
                    for kb in range(nkb):
                        r = kb - 4 * qc
                        o = max(r, 0) * 128
                        ps_s = psS.tile([128, 512], F32, tag="psS", name="ps_s")
                        nc.tensor.matmul(
                            ps_s[:, o:], lhsT=kt_sb[:, ts(kb, 128)],
                            rhs=qt_sb[:, h, 512 * qc + o:512 * (qc + 1)],
                            start=True, stop=True)
                        pt = work.tile([128, 512], BF, tag="pt", name="pt",
                                       bufs=6)
                        nc.scalar.activation(
                            pt[:, o:], ps_s[:, o:],
                            mybir.ActivationFunctionType.Exp, scale=SCALE)
                        if r >= 0:
                            nc.vector.tensor_tensor(
                                pt[:, o:], pt[:, o:],
                                mask_sb[:, :512 - o],
                                mybir.AluOpType.mult)
                        nc.tensor.matmul(
                            ps_att[:, o:], lhsT=v_sb[:, kb, :], rhs=pt[:, o:],
                            start=(kb == 0), stop=(kb == nkb - 1))
                        if kb == 0:
                            nc.vector.tensor_copy(acc, pt)
                        else:
                            nc.vector.tensor_tensor(
                                acc[:, o:], acc[:, o:], pt[:, o:],
                                mybir.AluOpType.add)
                    ps_den = psS.tile([128, 512], F32, tag="psS", name="ps_den")
                    nc.tensor.matmul(ps_den, lhsT=onesc_sb, rhs=acc,
                                     start=True, stop=True)
                    bden = work.tile([128, 512], F32, tag="bden", name="bden")
                    nc.vector.reciprocal_approx_fast(bden, ps_den)
                    att = work.tile([128, 512], BF, tag="att", name="att")
                    nc.vector.tensor_tensor(att, ps_att, bden,
                                            mybir.AluOpType.mult)
                    nc.scalar.dma_start(ag_in[qc][h // 2][ts(h % 2, 128), :], att)
                    if h % 2 == 1:
                        nc.gpsimd.collective_compute(
                            "AllGather", mybir.AluOpType.bypass,
                            replica_groups=RG,
                            ins=[ag_in[qc][h // 2][:].opt()],
                            outs=[ag_out[qc][h // 2][:].opt()])

            def oproj_chunk(tc_i):
                # two rhs tiles -> the j-chains' first 8 steps depend only on
                # the first pair-gather, so they overlap the last attention
                # blocks instead of waiting for both gathers
                rhsA = io.tile([128, 8, 512], BF, tag="io256", name="oproj_rA",
                               bufs=4)
                nc.sync.dma_start(
                    rhsA, ag_out[tc_i][0].rearrange("(o p) t -> p o t", p=128))
                rhsB = io.tile([128, 8, 512], BF, tag="io256", name="oproj_rB",
                               bufs=4)
                nc.sync.dma_start(
                    rhsB, ag_out[tc_i][1].rearrange("(o p) t -> p o t", p=128))
                for j in range(4):
                    ps_o = psA.tile([128, 512], F32, tag="psA", name="ps_o")
                    for c in range(ND):
                        rhs_c = rhsA[:, c, :] if c < 8 else rhsB[:, c - 8, :]
                        nc.tensor.matmul(
                            ps_o, lhsT=woT_sb[:, c, ts(j, 128)], rhs=rhs_c,
                            start=(c == 0), stop=(c == ND - 1))
                    o16 = work.tile([128, 512], BF, tag="o16", name="o16")
                    nc.vector.tensor_copy(o16, ps_o)
                    nc.sync.dma_start(out_d[ts(j, 128), ts(tc_i, 512)], o16)

            for i in range(NT):
                proj_chunk(i)
                attn_chunk(i)
            for i in range(NT):
                oproj_chunk(i)

    nc.compile()
    return nc


def make_in_maps(x, freqs_cos, freqs_sin, wq, wk, wv, wo):
    fc = np.asarray(freqs_cos, np.float32)
    fs = np.asarray(freqs_sin, np.float32)
    cos_exp = np.ascontiguousarray(np.repeat(fc.T, 2, axis=0)).astype(bf16)
    sgn = np.tile(np.array([-1.0, 1.0], np.float32), 64)[:, None]
    sin_sgn = np.ascontiguousarray(np.repeat(fs.T, 2, axis=0) * sgn).astype(bf16)
    mask01 = np.triu(np.ones((128, 512), np.float32), 0).astype(bf16)
    pswap = np.zeros((128, 128), np.float32)
    pswap[np.arange(128), np.arange(128) ^ 1] = 1.0
    pswap = pswap.astype(bf16)
    ident = np.eye(128, dtype=np.float32).astype(bf16)
    onesc = np.ones((128, 128), np.float32).astype(bf16)

    # x^T in chunk-major layout [chunk][p][o][t] so DMA lines are 4KB
    xt = []
    for b in range(B):
        t = np.ascontiguousarray(np.asarray(x[b], np.float32).T).astype(bf16)
        xt.append(np.ascontiguousarray(
            t.reshape(ND, 128, NT, 512).transpose(2, 1, 0, 3)))
    in_maps = []
    for core in range(8):
        b, g = divmod(core, 4)
        wqkvT = np.concatenate(
            [np.asarray(wk, np.float32)[128 * g:128 * (g + 1)].T,
             np.asarray(wv, np.float32)[128 * g:128 * (g + 1)].T,
             np.asarray(wq, np.float32)[512 * g:512 * (g + 1)].T], axis=1)
        wqkvT = np.ascontiguousarray(
            wqkvT.reshape(16, 128, 768).transpose(2, 1, 0)
        )
        wqkvT = np.ascontiguousarray(np.stack(
            [wqkvT[128 * m:128 * (m + 1)].transpose(1, 2, 0).reshape(128, 2048)
             for m in range(6)]))
        order = [0, 1, 4, 5, 8, 9, 12, 13, 2, 3, 6, 7, 10, 11, 14, 15]
        woT = np.asarray(wo, np.float32)[512 * g:512 * (g + 1), :].T
        woT = woT.reshape(16, 128, 512)[order].reshape(2048, 512)
        in_maps.append({
            "xt": xt[b],
            "wqkvT": np.ascontiguousarray(wqkvT).astype(bf16),
            "woT": np.ascontiguousarray(woT).astype(bf16),
            "cose": cos_exp,
            "sins": sin_sgn,
            "mask01": mask01,
            "pswap": pswap,
            "ident": ident,
            "onesc": onesc,
        })
    return in_maps


_NC = None


def get_nc():
    global _NC
    if _NC is None:
        _NC = build_nc()
    return _NC


def assemble_out(results):
    out = np.zeros((B, S, D), np.float32)
    for core in range(8):
        b, g = divmod(core, 4)
        out[b, :, 512 * g:512 * (g + 1)] = np.asarray(results[core]["out"], np.float32).T
    return out


def kernel(x, freqs_cos, freqs_sin, wq, wk, wv, wo):
    import os
    os.environ.setdefault("BASS_NEVER_TRACE", "1")  # NTFF hook absent headless
    nc = get_nc()
    in_maps = make_in_maps(x, freqs_cos, freqs_sin, wq, wk, wv, wo)
    res = run_bass_kernel_spmd(nc, in_maps, core_ids=list(range(8)))
    return assemble_out(res.results)
